# revision 1
# baseline (speedup 1.0000x reference)
"""Trainium2 Bass kernel for nn_CircumpunctAttention_17136919511703.

Sharding: 16 heads tensor-parallel over 8 cores (2 heads/core); W_out
row-parallel with the cross-core partial sum done on the host.

Math simplifications (validated to ~2e-7 abs err vs the jax reference,
output absmax ~0.03):
  - attn = softmax(s)*ap*ex renormalized by (sum + 1e-8): the aperture
    gate ap cancels exactly up to the 1e-8 term (relative ~4e-8), so it
    is dropped entirely.  converged = (e' @ v) / (e' @ 1) with
    e'_st = exp(scoresT_st + ln(ex_s)) (sender gate folded into the exp
    bias, per-partition on the ACT engine).
  - no softmax max-subtraction (scores are bounded, |s| < 4).
  - the per-head "aperture chamber" (valve in/out, phase rotation, chi)
    is a per-head linear map on the head dim -> folded into W_out on the
    host: W'_h = c_h * Wout_h @ R_h.

Per-core dataflow (all matmuls fp32r):
  xT [D,T] (host-transposed) -> innerT/outerT/vT [128,T] + ex logits
  [2,T] projections; v transposed back to natural via PE transposes to
  build vpp=[v|1] [128, t_tile, head, 65]; per head: scoresT [s,t] in
  PSUM -> ACT exp(+lnex bias) -> eT SBUF -> PT accumulation
  [65(=64+E row), T] -> f=1/E -> conv scaled via a K=1 broadcast matmul
  -> final y_partial = convT.T @ wpT -> DMA out.  Host sums 8 partials.
"""

import math
import os
from contextlib import ExitStack

import numpy as np

import concourse.bass as bass
import concourse.mybir as mybir
import concourse.tile as tile
from concourse import bacc
from concourse.bass_utils import run_bass_kernel_spmd
from concourse.masks import make_identity

T, D, H, DH = 2048, 1024, 16, 64
NCORES = 8
HPC = H // NCORES          # heads per core = 2
KW = HPC * DH              # per-core head width = 128
SCALE = math.sqrt(DH)
DT = D // 128              # d tiles = 8
TT = T // 128              # t/s tiles = 16
CH = T // 512              # 512-col chunks over T = 4
F32 = mybir.dt.float32
F32R = mybir.dt.float32r
AF = mybir.ActivationFunctionType

_CACHE = {}
LAST_RESULTS = None


def _build_nc():
    nc = bacc.Bacc()
    xT = nc.declare_dram_parameter("xT", [128, DT, T], F32R, isOutput=False)
    wiT = nc.declare_dram_parameter("wiT", [128, DT, KW], F32R, isOutput=False)
    woT = nc.declare_dram_parameter("woT", [128, DT, KW], F32R, isOutput=False)
    wvT = nc.declare_dram_parameter("wvT", [128, DT, KW], F32R, isOutput=False)
    weT = nc.declare_dram_parameter("weT", [128, DT, HPC], F32R, isOutput=False)
    webn = nc.declare_dram_parameter("webn", [128, TT, HPC], F32, isOutput=False)
    wpT = nc.declare_dram_parameter("wpT", [KW, D], F32R, isOutput=False)
    ones2 = nc.declare_dram_parameter("ones2", [128, 64], F32R, isOutput=False)
    y = nc.declare_dram_parameter("y", [T, D], F32, isOutput=True)

    with tile.TileContext(nc) as tc, ExitStack() as ctx:
        _body(ctx, tc, xT, wiT, woT, wvT, weT, webn, wpT, ones2, y)
    nc.compile()
    return nc


def _body(ctx, tc, xT, wiT, woT, wvT, weT, webn, wpT, ones2, y):
    nc = tc.nc
    P = 128
    HT = 1024  # t-half width

    const = ctx.enter_context(tc.tile_pool(name="const", bufs=1))
    persist = ctx.enter_context(tc.tile_pool(name="persist", bufs=1))
    eTp = ctx.enter_context(tc.tile_pool(name="eTp", bufs=10))
    stage = ctx.enter_context(tc.tile_pool(name="stage", bufs=2))
    # single uniform PSUM pool: 4 slots x [128, 1024] (2 banks each)
    psum = ctx.enter_context(tc.tile_pool(name="psum", bufs=4, space="PSUM"))

    def ps_tile(name="psm"):
        return psum.tile([P, HT], F32, tag="m", name=name)

    ident = const.tile([P, P], F32)
    make_identity(nc, ident)
    ones_k1 = const.tile([1, 64], F32R)
    nc.sync.dma_start(out=ones_k1, in_=ones2[0:1, :])
    onep = const.tile([P, 1], F32)
    nc.vector.memset(onep, 1.0)
    bnat = const.tile([P, TT, HPC], F32)
    nc.sync.dma_start(out=bnat, in_=webn[:, :, :])

    # weights (d on partitions): [p, d_tile, k]
    wiT_sb = const.tile([P, DT, KW], F32R)
    woT_sb = const.tile([P, DT, KW], F32R)
    wvT_sb = const.tile([P, DT, KW], F32R)
    weT_sb = const.tile([P, DT, HPC], F32R)
    nc.sync.dma_start(out=wiT_sb, in_=wiT[:, :, :])
    nc.sync.dma_start(out=woT_sb, in_=woT[:, :, :])
    nc.sync.dma_start(out=wvT_sb, in_=wvT[:, :, :])
    nc.sync.dma_start(out=weT_sb, in_=weT[:, :, :])
    wpT_sb = const.tile([KW, D], F32R)
    nc.sync.dma_start(out=wpT_sb, in_=wpT[:, :])

    xT_sb = persist.tile([P, DT, T], F32R)
    for a in range(DT):
        nc.sync.dma_start(out=xT_sb[:, a, :], in_=xT[:, a, :])

    innerT = persist.tile([P, T], F32R)
    outerT = persist.tile([P, T], F32R)
    vT = persist.tile([P, T], F32)
    exlT = persist.tile([HPC, T], F32)
    vpp = persist.tile([P, TT, HPC, 65], F32R)
    nc.sync.dma_start(
        out=vpp[:, :, :, 64],
        in_=ones2[:, 0:TT * HPC].rearrange("p (a b) -> p a b", a=TT))
    ex_nat = persist.tile([P, TT, HPC], F32)
    convT = persist.tile([KW, T], F32R)

    def xchunk(a, half, j2):
        base = half * HT + j2 * 512
        return xT_sb[:, a, base:base + 512].bitcast(F32R)

    # ---- projection wave 1 (a-major, all 4 psum slots in flight so the
    # PE consumes each xT d-tile as its DMA lands) ----
    psA, psB = ps_tile("ex0"), ps_tile("v0")
    ps1, ps2 = ps_tile("i0"), ps_tile("o0")
    for a in range(DT):
        st, sp_ = (a == 0), (a == DT - 1)
        for j2 in range(2):
            nc.tensor.matmul(
                psA[0:HPC, j2 * 512:(j2 + 1) * 512],
                lhsT=weT_sb[:, a, :].bitcast(F32R),
                rhs=xchunk(a, 0, j2), start=st, stop=sp_)
            nc.tensor.matmul(
                psB[:, j2 * 512:(j2 + 1) * 512],
                lhsT=wvT_sb[:, a, :].bitcast(F32R),
                rhs=xchunk(a, 0, j2), start=st, stop=sp_)
            nc.tensor.matmul(
                ps1[:, j2 * 512:(j2 + 1) * 512],
                lhsT=wiT_sb[:, a, :].bitcast(F32R),
                rhs=xchunk(a, 0, j2), start=st, stop=sp_)
            nc.tensor.matmul(
                ps2[:, j2 * 512:(j2 + 1) * 512],
                lhsT=woT_sb[:, a, :].bitcast(F32R),
                rhs=xchunk(a, 0, j2), start=st, stop=sp_)
    nc.scalar.copy(out=exlT[:, 0:HT], in_=psA[0:HPC, :])
    nc.scalar.copy(out=vT[:, 0:HT], in_=psB)
    nc.vector.tensor_copy(out=innerT[:, 0:HT], in_=ps1)
    nc.vector.tensor_copy(out=outerT[:, 0:HT], in_=ps2)

    def emit_proj(wsb, half, rows=P, name="pw"):
        ps = ps_tile(name)
        for a in range(DT):
            for j2 in range(2):
                nc.tensor.matmul(
                    ps[0:rows, j2 * 512:(j2 + 1) * 512],
                    lhsT=wsb[:, a, :].bitcast(F32R),
                    rhs=xchunk(a, half, j2),
                    start=(a == 0), stop=(a == DT - 1))
        return ps

    def emit_sig(half):
        # ex_nat[:, i, h] = sigmoid(z + b), Exp-only (one ACT table set)
        ps = ps_tile("sig")
        for k in range(8):
            i = half * 8 + k
            nc.tensor.transpose(
                ps[:, k * 64:k * 64 + HPC], exlT[:, i * P:(i + 1) * P],
                ident[0:HPC, 0:HPC])
        sl = slice(half * 8, (half + 1) * 8)
        nc.vector.tensor_add(
            out=ex_nat[:, sl, :],
            in0=ps.rearrange("p (i c) -> p i c", c=64)[:, 0:8, 0:HPC],
            in1=bnat[:, sl, :])
        nc.scalar.activation(out=ex_nat[:, sl, :], in_=ex_nat[:, sl, :],
                             func=AF.Exp, scale=-1.0)
        nc.vector.tensor_scalar(
            out=ex_nat[:, sl, :], in0=ex_nat[:, sl, :],
            scalar1=onep, scalar2=None, op0=mybir.AluOpType.add)
        nc.vector.reciprocal(out=ex_nat[:, sl, :], in_=ex_nat[:, sl, :])

    def emit_vppg(g):
        # vpp[:, i, h, :] = [v_i_h | 1] * ex_nat[:, i, h]
        ps = ps_tile("vtr")
        for k in range(8):
            i = g * 8 + k
            nc.tensor.transpose(
                ps[:, k * P:(k + 1) * P], vT[:, i * P:(i + 1) * P], ident)
        psr = ps.rearrange("p (k c) -> p k c", c=P)
        for h in range(HPC):
            nc.vector.tensor_copy(
                out=vpp[:, g * 8:(g + 1) * 8, h, 0:DH],
                in_=psr[:, :, h * DH:(h + 1) * DH])
        for k in range(8):
            i = g * 8 + k
            for h in range(HPC):
                nc.vector.tensor_scalar_mul(
                    out=vpp[:, i, h, :], in0=vpp[:, i, h, :],
                    scalar1=ex_nat[:, i, h:h + 1])

    emit_sig(0)
    emit_vppg(0)

    convT = persist.tile([KW, T], F32R)

    # ---- attention, both heads packed per i: scores h0/h1 run in PE row
    # groups (0,0)/(64,0) into one psum tile, one wide exp covers both ----
    def emit_attn2(i, jj, pt0, pt1):
        for j2 in range(2):
            j = jj * 2 + j2
            sc = ps_tile("sc")
            nc.tensor.matmul(
                sc[:, 0:512],
                lhsT=outerT[0:DH, i * P:(i + 1) * P].bitcast(F32R),
                rhs=innerT[0:DH, j * 512:(j + 1) * 512].bitcast(F32R),
                start=True, stop=True)
            nc.tensor.matmul(
                sc[:, 512:HT],
                lhsT=outerT[DH:KW, i * P:(i + 1) * P].bitcast(F32R),
                rhs=innerT[DH:KW, j * 512:(j + 1) * 512].bitcast(F32R),
                start=True, stop=True)
            eT = eTp.tile([P, HT], F32R, tag="e", name="eT")
            nc.scalar.activation(out=eT, in_=sc, func=AF.Exp, scale=1.0)
            nc.tensor.matmul(
                pt0[0:65, j2 * 512:(j2 + 1) * 512],
                lhsT=vpp[:, i, 0, :].bitcast(F32R),
                rhs=eT[:, 0:512].bitcast(F32R),
                start=(i == 0), stop=(i == TT - 1))
            nc.tensor.matmul(
                pt1[0:65, j2 * 512:(j2 + 1) * 512],
                lhsT=vpp[:, i, 1, :].bitcast(F32R),
                rhs=eT[:, 512:HT].bitcast(F32R),
                start=(i == 0), stop=(i == TT - 1))

    def emit_post(h, jj, pt):
        # conv[:, half] = PT[0:64] / E ; via f=1/E broadcast K=1 matmul
        f_row = stage.tile([HPC, HT], F32R, tag="sp", name="f_row")[0:1, :]
        with nc.allow_low_precision(reason="f32r is a 4-byte container"):
            nc.vector.reciprocal(out=f_row, in_=pt[64:65, 0:HT])
        p_sb = stage.tile([DH, HT], F32, tag="P")
        nc.vector.tensor_copy(out=p_sb[:, 0:512], in_=pt[0:DH, 0:512])
        nc.scalar.copy(out=p_sb[:, 512:HT], in_=pt[0:DH, 512:HT])
        fps = ps_tile("fps")
        for j2 in range(2):
            nc.tensor.matmul(
                fps[0:DH, j2 * 512:(j2 + 1) * 512],
                lhsT=ones_k1.bitcast(F32R),
                rhs=f_row[:, j2 * 512:(j2 + 1) * 512].bitcast(F32R),
                start=True, stop=True)
        nc.vector.tensor_mul(
            out=convT[h * DH:(h + 1) * DH, jj * HT:(jj + 1) * HT],
            in0=p_sb, in1=fps[0:DH, :])

    def emit_final_slice(m, dve_only=False):
        py = ps_tile("py")
        for o in range(2):
            nc.tensor.matmul(
                py[:, o * 512:(o + 1) * 512],
                lhsT=convT[:, m * P:(m + 1) * P].bitcast(F32R),
                rhs=wpT_sb[:, o * 512:(o + 1) * 512].bitcast(F32R),
                start=True, stop=True)
        y_sb = stage.tile([P, D], F32, tag="y", bufs=3, name="y_sb")
        if dve_only or m % 2 == 0:
            nc.vector.tensor_copy(out=y_sb, in_=py)
        else:
            nc.scalar.copy(out=y_sb, in_=py)
        nc.sync.dma_start(out=y[m * P:(m + 1) * P, :], in_=y_sb)

    # wave-2 groups, interleaved into the first attention phase
    def w2_ex1():
        ps = emit_proj(weT_sb, 1, rows=HPC, name="ex1")
        nc.scalar.copy(out=exlT[:, HT:T], in_=ps[0:HPC, :])
        emit_sig(1)

    def w2_v1():
        ps = emit_proj(wvT_sb, 1, name="v1")
        nc.scalar.copy(out=vT[:, HT:T], in_=ps)
        emit_vppg(1)

    def w2_o1():
        ps = emit_proj(woT_sb, 1, name="o1")
        nc.vector.tensor_copy(out=outerT[:, HT:T], in_=ps)

    def w2_i1():
        ps = emit_proj(wiT_sb, 1, name="i1")
        nc.vector.tensor_copy(out=innerT[:, HT:T], in_=ps)

    wave2 = [w2_ex1, w2_v1, w2_o1, w2_i1]

    # jj=0 for both heads; wave-2 trickles through the spare psum slot
    pt00 = psum.tile([P, HT], F32, tag="m", name="pt00")
    pt10 = psum.tile([P, HT], F32, tag="m", name="pt10")
    for i in range(8):
        emit_attn2(i, 0, pt00, pt10)
        if i % 2 == 1:
            wave2[i // 2]()
    for i in range(8, TT):
        emit_attn2(i, 0, pt00, pt10)
    emit_post(0, 0, pt00)
    emit_post(1, 0, pt10)

    # jj=1 for both heads; final slices of t-half 0 interleave in
    pt01 = psum.tile([P, HT], F32, tag="m", name="pt01")
    pt11 = psum.tile([P, HT], F32, tag="m", name="pt11")
    for i in range(TT):
        emit_attn2(i, 1, pt01, pt11)
        if 2 <= i < 10:
            emit_final_slice(i - 2, dve_only=True)
    emit_post(0, 1, pt01)
    emit_post(1, 1, pt11)
    for m in range(8, TT):
        emit_final_slice(m)


def _sigmoid(z):
    return 1.0 / (1.0 + np.exp(-z))


def _prep_in_maps(inputs):
    x = np.ascontiguousarray(np.asarray(inputs["x"], np.float32)[0])  # [T, D]
    xT = np.ascontiguousarray(x.T)                                    # [D, T]
    Wi = np.asarray(inputs["Wi_w"], np.float32).reshape(H, DH, D) / SCALE
    Wo = np.asarray(inputs["Wo_w"], np.float32).reshape(H, DH, D)
    Wv = np.asarray(inputs["Wv_w"], np.float32).reshape(H, DH, D)
    We = np.asarray(inputs["We_w"], np.float32)                       # [H, D]
    We_b = np.asarray(inputs["We_b"], np.float32)                     # [H]
    Wout = np.asarray(inputs["Wout_w"], np.float32)                   # [D, D]
    beta = np.asarray(inputs["beta"], np.float32)
    iv = np.asarray(inputs["iv"], np.float32)
    ov = np.asarray(inputs["ov"], np.float32)
    chi = np.asarray(inputs["chi"], np.float32)

    # chamber folded into Wout: W'_h = c_h * Wout_h @ R_h
    ang = np.float32(math.pi) * _sigmoid(beta)
    c_h = _sigmoid(iv) * _sigmoid(ov) * np.tanh(chi)                  # [H]
    cos_a, sin_a = np.cos(ang), np.sin(ang)
    HALF = DH // 2
    Wp = np.zeros((H, D, DH), np.float32)
    for h in range(H):
        Wh = Wout[:, h * DH:(h + 1) * DH]
        Wp[h][:, :HALF] = c_h[h] * (Wh[:, :HALF] * cos_a[h] + Wh[:, HALF:] * sin_a[h])
        Wp[h][:, HALF:] = c_h[h] * (-Wh[:, :HALF] * sin_a[h] + Wh[:, HALF:] * cos_a[h])

    def dtile(arr):  # [D, X] -> [128, DT, X] (d-tile-major, partition-contig)
        return np.ascontiguousarray(
            arr.reshape(DT, 128, arr.shape[1]).transpose(1, 0, 2))

    xTr = dtile(xT)
    ones2 = np.ones((128, 64), np.float32)
    in_maps = []
    for c in range(NCORES):
        hs = slice(HPC * c, HPC * (c + 1))
        wiT = dtile(Wi[hs].reshape(KW, D).T)
        woT = dtile(Wo[hs].reshape(KW, D).T)
        wvT = dtile(Wv[hs].reshape(KW, D).T)
        weT = dtile(We[hs].T)
        webn = np.ascontiguousarray(
            np.broadcast_to(We_b[hs], (128, TT, HPC)).astype(np.float32))
        wpT = np.ascontiguousarray(
            Wp[hs].transpose(0, 2, 1).reshape(KW, D))                 # [128, D]
        in_maps.append(dict(xT=xTr, wiT=wiT, woT=woT, wvT=wvT,
                            weT=weT, webn=webn, wpT=wpT, ones2=ones2))
    return in_maps


def kernel(**inputs):
    global LAST_RESULTS
    if "nc" not in _CACHE:
        _CACHE["nc"] = _build_nc()
    nc = _CACHE["nc"]
    in_maps = _prep_in_maps(inputs)
    trace = bool(os.environ.get("CIRC_TRACE"))
    res = run_bass_kernel_spmd(
        nc, in_maps, core_ids=list(range(NCORES)), trace=trace)
    LAST_RESULTS = res
    y = res.results[0]["y"].astype(np.float32)
    for c in range(1, NCORES):
        y = y + res.results[c]["y"]
    return y.reshape(1, T, D)



# revision 10
# speedup vs baseline: 1.3876x; 1.3876x over previous
"""Trainium2 Bass kernel for nn_CircumpunctAttention_17136919511703.

Sharding: 16 heads tensor-parallel over 8 cores (2 heads/core); W_out
row-parallel with the cross-core partial sum done on the host.

Math simplifications (validated vs the jax reference):
  - attn = softmax(s)*ap*ex renormalized by (sum + 1e-8): the aperture
    gate ap cancels exactly up to the 1e-8 term, so it is dropped.
    converged = (e' @ v) / (e' @ 1) with e'_st = ex_s * exp(scoresT_st)
    (sender gate folded into vpp).
  - no softmax max-subtraction (scores are bounded, |s| < 4).
  - the per-head "aperture chamber" (valve in/out, phase rotation, chi)
    is a per-head linear map on the head dim -> folded into W_out on the
    host: W'_h = c_h * Wout_h @ R_h.

Schedule (engines execute strictly in program order, so emission order
is the schedule):
  wave(half): i/o/v/ex projections for one T-half, 4 concurrent PSUM
    accumulation streams paced to the xT tile DMAs; ex is computed in
    natural [t, h] orientation (tiny free dim) so no transposes or M=2
    waves are needed; v transposed to natural and gated by ex into vpp.
  attention: 4 j-blocks of 512 t-cols; per block a software-pipelined
    i-loop emits scores(i+2)+exp(i+2) BEFORE PT(i) so the PE never
    waits on the ACT exp; pt accumulators [65, 512] live in one 2-bank
    PSUM slot.  post(j) normalizes via DVE reciprocal + Pool
    partition-broadcast + DVE multiply (no PE, no ACT).  Final output
    matmuls for block j are interleaved into block j+1's loop; y tiles
    staged via Pool-engine copies and DMA'd out per 128-row slice.
PSUM budget: tags sc(2x[128,1024]) + pt(1x[128,1024]) + py(2x[128,512])
  = exactly 8 banks, shared by all phases.
"""

import math
import os
from contextlib import ExitStack

import numpy as np

import concourse.bass as bass
import concourse.mybir as mybir
import concourse.tile as tile
from concourse import bacc
from concourse.bass_utils import run_bass_kernel_spmd
from concourse.masks import make_identity

T, D, H, DH = 2048, 1024, 16, 64
NCORES = 8
HPC = H // NCORES          # heads per core = 2
KW = HPC * DH              # per-core head width = 128
SCALE = math.sqrt(DH)
DT = D // 128              # d tiles = 8
TT = T // 128              # t/s tiles = 16
F32 = mybir.dt.float32
F32R = mybir.dt.float32r
AF = mybir.ActivationFunctionType

_CACHE = {}
LAST_RESULTS = None


def _build_nc():
    nc = bacc.Bacc()
    xT = nc.declare_dram_parameter("xT", [128, DT, T], F32R, isOutput=False)
    wiT = nc.declare_dram_parameter("wiT", [128, DT, KW], F32R, isOutput=False)
    woT = nc.declare_dram_parameter("woT", [128, DT, KW], F32R, isOutput=False)
    wvT = nc.declare_dram_parameter("wvT", [128, DT, KW], F32R, isOutput=False)
    weT = nc.declare_dram_parameter("weT", [128, DT, HPC], F32R, isOutput=False)
    webn = nc.declare_dram_parameter("webn", [128, TT * HPC], F32, isOutput=False)
    wpT = nc.declare_dram_parameter("wpT", [KW, D], F32R, isOutput=False)
    y = nc.declare_dram_parameter("y", [T, D], F32, isOutput=True)

    with tile.TileContext(nc) as tc, ExitStack() as ctx:
        _body(ctx, tc, xT, wiT, woT, wvT, weT, webn, wpT, y)
    nc.compile()
    return nc


def _body(ctx, tc, xT, wiT, woT, wvT, weT, webn, wpT, y):
    nc = tc.nc
    P = 128
    HB = 512                   # j-block width
    NJ = T // HB               # 4 j-blocks

    const = ctx.enter_context(tc.tile_pool(name="const", bufs=1))
    persist = ctx.enter_context(tc.tile_pool(name="persist", bufs=1))
    eTp = ctx.enter_context(tc.tile_pool(name="eTp", bufs=4))
    stage = ctx.enter_context(tc.tile_pool(name="stage", bufs=2))
    psum = ctx.enter_context(tc.tile_pool(name="psum", bufs=1, space="PSUM"))

    # ---- DMA issue order == consumption order ----
    wiT_sb = const.tile([P, DT, KW], F32R)
    woT_sb = const.tile([P, DT, KW], F32R)
    wvT_sb = const.tile([P, DT, KW], F32R)
    weT_sb = const.tile([P, DT, HPC], F32R)
    xT_sb = persist.tile([P, DT, T], F32R)
    wpT_sb = const.tile([KW, D], F32R)
    webn_sb = const.tile([P, TT * HPC], F32)
    nc.sync.dma_start(out=wiT_sb, in_=wiT[:, :, :])
    nc.sync.dma_start(out=xT_sb[:, 0, 0:1024], in_=xT[:, 0, 0:1024])
    nc.sync.dma_start(out=woT_sb, in_=woT[:, :, :])
    nc.sync.dma_start(out=wvT_sb, in_=wvT[:, :, :])
    nc.sync.dma_start(out=weT_sb, in_=weT[:, :, :])
    for a in range(1, DT):
        nc.sync.dma_start(out=xT_sb[:, a, 0:1024], in_=xT[:, a, 0:1024])
    nc.sync.dma_start(out=webn_sb, in_=webn[:, :])
    for a in range(DT):
        nc.sync.dma_start(out=xT_sb[:, a, 1024:T], in_=xT[:, a, 1024:T])
    nc.sync.dma_start(out=wpT_sb, in_=wpT[:, :])

    ident = const.tile([P, P], F32)
    make_identity(nc, ident)
    onep = const.tile([P, 1], F32)
    nc.vector.memset(onep, 1.0)

    innerT = persist.tile([P, T], F32R)
    outerT = persist.tile([P, T], F32R)
    vT = persist.tile([P, T], F32)
    vpp = persist.tile([P, TT, HPC, 65], F32R)
    # col 64 must be 1.0 (ones row for the E sums); cols 0:64 are fully
    # overwritten by the v copies, so a contiguous whole-tile memset works
    nc.vector.memset(vpp.bitcast(F32), 1.0)
    exz = persist.tile([P, TT * HPC], F32)   # sigmoid(ex logits), [t, i*2+h]
    convT = persist.tile([KW, T], F32R)

    def xchunk(a, c):
        # c indexes 512-wide chunks of T
        return xT_sb[:, a, c * 512:(c + 1) * 512].bitcast(F32R)

    def wave(half):
        psI = psum.tile([P, 1024], F32, tag="sc", bufs=2, name="psI")
        psO = psum.tile([P, 1024], F32, tag="sc", bufs=2, name="psO")
        psV = psum.tile([P, 1024], F32, tag="pt", bufs=1, name="psV")
        for a in range(DT):
            st, sp = (a == 0), (a == DT - 1)
            for j2 in range(2):
                c = half * 2 + j2
                nc.tensor.matmul(
                    psI[:, j2 * 512:(j2 + 1) * 512],
                    lhsT=wiT_sb[:, a, :].bitcast(F32R), rhs=xchunk(a, c),
                    start=st, stop=sp)
                nc.tensor.matmul(
                    psO[:, j2 * 512:(j2 + 1) * 512],
                    lhsT=woT_sb[:, a, :].bitcast(F32R), rhs=xchunk(a, c),
                    start=st, stop=sp)
                nc.tensor.matmul(
                    psV[:, j2 * 512:(j2 + 1) * 512],
                    lhsT=wvT_sb[:, a, :].bitcast(F32R), rhs=xchunk(a, c),
                    start=st, stop=sp)
        hs = slice(half * 1024, (half + 1) * 1024)
        nc.vector.tensor_copy(out=innerT[:, hs], in_=psI)
        nc.vector.tensor_copy(out=outerT[:, hs], in_=psO)
        nc.scalar.copy(out=vT[:, hs], in_=psV)

        # ex logits per t-tile (natural [t, h] layout, all tiny); one
        # accumulation group per PSUM bank (start=True resets bank-wide)
        for k in range(8):
            m = half * 8 + k
            pex = psum.tile([P, HPC], F32, tag="py", bufs=2, name="pex")
            for a in range(DT):
                nc.tensor.matmul(
                    pex,
                    lhsT=xT_sb[:, a, m * P:(m + 1) * P].bitcast(F32R),
                    rhs=weT_sb[:, a, :].bitcast(F32R),
                    start=(a == 0), stop=(a == DT - 1))
            nc.vector.tensor_add(out=exz[:, HPC * m:HPC * (m + 1)], in0=pex,
                                 in1=webn_sb[:, HPC * m:HPC * (m + 1)])
        cs = slice(half * 16, (half + 1) * 16)
        nc.scalar.activation(out=exz[:, cs], in_=exz[:, cs],
                             func=AF.Exp, scale=-1.0)
        nc.vector.tensor_scalar(out=exz[:, cs], in0=exz[:, cs],
                                scalar1=onep, scalar2=None,
                                op0=mybir.AluOpType.add)
        nc.vector.reciprocal(out=exz[:, cs], in_=exz[:, cs])

        # v -> natural per-head layout, gated by ex
        pstr = psum.tile([P, 1024], F32, tag="sc", bufs=2, name="pstr")
        for k in range(8):
            i = half * 8 + k
            nc.tensor.transpose(
                pstr[:, k * P:(k + 1) * P], vT[:, i * P:(i + 1) * P], ident)
        psr = pstr.rearrange("p (k c) -> p k c", c=P)
        g8 = slice(half * 8, (half + 1) * 8)
        for h in range(HPC):
            nc.vector.tensor_copy(
                out=vpp[:, g8, h, 0:DH],
                in_=psr[:, :, h * DH:(h + 1) * DH])
        for k in range(8):
            i = half * 8 + k
            for h in range(HPC):
                nc.vector.tensor_scalar_mul(
                    out=vpp[:, i, h, :], in0=vpp[:, i, h, :],
                    scalar1=exz[:, HPC * i + h:HPC * i + h + 1])

    wave(0)
    wave(1)

    # ---- attention: one flat software-pipelined loop over k = j*16+i ----
    NK = NJ * TT               # 64

    def sc_exp(k):
        i, j = k % TT, k // TT
        sc = psum.tile([P, 1024], F32, tag="sc", bufs=2, name="sc")
        nc.tensor.matmul(
            sc[:, 0:512],
            lhsT=outerT[0:DH, i * P:(i + 1) * P].bitcast(F32R),
            rhs=innerT[0:DH, j * HB:(j + 1) * HB].bitcast(F32R),
            start=True, stop=True)
        nc.tensor.matmul(
            sc[:, 512:1024],
            lhsT=outerT[DH:KW, i * P:(i + 1) * P].bitcast(F32R),
            rhs=innerT[DH:KW, j * HB:(j + 1) * HB].bitcast(F32R),
            start=True, stop=True)
        eT = eTp.tile([P, 1024], F32R, tag="e", name="eT")
        nc.scalar.activation(out=eT, in_=sc, func=AF.Exp, scale=1.0)
        return eT

    def emit_final(m, tail=False):
        # y partial for t-tile m; PSUM->SBUF copies on DVE (ACT helps in the
        # tail, once the exp stream is done), then DMA
        y_sb = stage.tile([P, D], F32, tag="y", name="y_sb")
        for o in range(2):
            py = psum.tile([P, 512], F32, tag="py", bufs=2, name="py")
            nc.tensor.matmul(
                py,
                lhsT=convT[:, m * P:(m + 1) * P].bitcast(F32R),
                rhs=wpT_sb[:, o * 512:(o + 1) * 512].bitcast(F32R),
                start=True, stop=True)
            if tail and o == 0:
                nc.scalar.copy(out=y_sb[:, 0:512], in_=py)
            else:
                nc.vector.tensor_copy(out=y_sb[:, o * 512:(o + 1) * 512],
                                      in_=py)
        nc.sync.dma_start(out=y[m * P:(m + 1) * P, :], in_=y_sb)

    def post(j, pt):
        # conv = PT[0:64] / E, E in row 64; no PE, no ACT.  One fast DVE
        # copy frees the pt PSUM slot; the rest runs from SBUF off the
        # critical path.
        pt_sb = stage.tile([65, 1024], F32, tag="ptsb", name="pt_sb")
        nc.vector.tensor_copy(out=pt_sb, in_=pt)
        f_row = stage.tile([1, 1024], F32R, tag="f", name="f_row")
        with nc.allow_low_precision(reason="f32r is a 4-byte container"):
            nc.vector.reciprocal(out=f_row, in_=pt_sb[64:65, 0:1024])
        fbc = stage.tile([DH, 1024], F32R, tag="fb", name="fbc")
        nc.gpsimd.partition_broadcast(fbc, f_row, channels=DH)
        jc = slice(j * HB, (j + 1) * HB)
        nc.vector.tensor_mul(out=convT[0:DH, jc], in0=pt_sb[0:DH, 0:512],
                             in1=fbc[:, 0:512])
        nc.vector.tensor_mul(out=convT[DH:KW, jc], in0=pt_sb[0:DH, 512:1024],
                             in1=fbc[:, 512:1024])

    y_pend = []                # finals not yet emitted: list of m
    pt = None
    eTs = {0: sc_exp(0), 1: sc_exp(1)}
    for k in range(NK):
        i, j = k % TT, k // TT
        if k + 2 < NK:
            eTs[k + 2] = sc_exp(k + 2)
        if i == 0:
            pt = psum.tile([65, 1024], F32, tag="pt", bufs=1, name="pt")
        if i in (4, 7, 10, 13) and y_pend:
            emit_final(y_pend.pop(0))
        eT = eTs.pop(k)
        nc.tensor.matmul(
            pt[0:65, 0:512],
            lhsT=vpp[:, i, 0, :].bitcast(F32R),
            rhs=eT[:, 0:512],
            start=(i == 0), stop=(i == TT - 1))
        nc.tensor.matmul(
            pt[0:65, 512:1024],
            lhsT=vpp[:, i, 1, :].bitcast(F32R),
            rhs=eT[:, 512:1024],
            start=(i == 0), stop=(i == TT - 1))
        if i == TT - 1:
            post(j, pt)
            y_pend.extend(range(j * 4, (j + 1) * 4))

    for m in y_pend:
        emit_final(m, tail=True)


def _sigmoid(z):
    return 1.0 / (1.0 + np.exp(-z))


def _prep_in_maps(inputs):
    x = np.ascontiguousarray(np.asarray(inputs["x"], np.float32)[0])  # [T, D]
    xT = np.ascontiguousarray(x.T)                                    # [D, T]
    Wi = np.asarray(inputs["Wi_w"], np.float32).reshape(H, DH, D) / SCALE
    Wo = np.asarray(inputs["Wo_w"], np.float32).reshape(H, DH, D)
    Wv = np.asarray(inputs["Wv_w"], np.float32).reshape(H, DH, D)
    We = np.asarray(inputs["We_w"], np.float32)                       # [H, D]
    We_b = np.asarray(inputs["We_b"], np.float32)                     # [H]
    Wout = np.asarray(inputs["Wout_w"], np.float32)                   # [D, D]
    beta = np.asarray(inputs["beta"], np.float32)
    iv = np.asarray(inputs["iv"], np.float32)
    ov = np.asarray(inputs["ov"], np.float32)
    chi = np.asarray(inputs["chi"], np.float32)

    # chamber folded into Wout: W'_h = c_h * Wout_h @ R_h
    ang = np.float32(math.pi) * _sigmoid(beta)
    c_h = _sigmoid(iv) * _sigmoid(ov) * np.tanh(chi)                  # [H]
    cos_a, sin_a = np.cos(ang), np.sin(ang)
    HALF = DH // 2
    Wp = np.zeros((H, D, DH), np.float32)
    for h in range(H):
        Wh = Wout[:, h * DH:(h + 1) * DH]
        Wp[h][:, :HALF] = c_h[h] * (Wh[:, :HALF] * cos_a[h] + Wh[:, HALF:] * sin_a[h])
        Wp[h][:, HALF:] = c_h[h] * (-Wh[:, :HALF] * sin_a[h] + Wh[:, HALF:] * cos_a[h])

    def dtile(arr):  # [D, X] -> [128, DT, X] (d-tile-major, partition-contig)
        return np.ascontiguousarray(
            arr.reshape(DT, 128, arr.shape[1]).transpose(1, 0, 2))

    xTr = dtile(xT)
    in_maps = []
    for c in range(NCORES):
        hs = slice(HPC * c, HPC * (c + 1))
        wiT = dtile(Wi[hs].reshape(KW, D).T)
        woT = dtile(Wo[hs].reshape(KW, D).T)
        wvT = dtile(Wv[hs].reshape(KW, D).T)
        weT = dtile(We[hs].T)
        webn = np.ascontiguousarray(np.broadcast_to(
            np.tile(We_b[hs], TT), (128, TT * HPC)).astype(np.float32))
        wpT = np.ascontiguousarray(
            Wp[hs].transpose(0, 2, 1).reshape(KW, D))                 # [128, D]
        in_maps.append(dict(xT=xTr, wiT=wiT, woT=woT, wvT=wvT,
                            weT=weT, webn=webn, wpT=wpT))
    return in_maps


def kernel(**inputs):
    global LAST_RESULTS
    if "nc" not in _CACHE:
        _CACHE["nc"] = _build_nc()
    nc = _CACHE["nc"]
    in_maps = _prep_in_maps(inputs)
    trace = os.environ.get("CIRC_TRACE", "") not in ("", "0")
    res = run_bass_kernel_spmd(
        nc, in_maps, core_ids=list(range(NCORES)), trace=trace)
    LAST_RESULTS = res
    y = res.results[0]["y"].astype(np.float32)
    for c in range(1, NCORES):
        y = y + res.results[c]["y"]
    return y.reshape(1, T, D)


# revision 12
# speedup vs baseline: 1.4572x; 1.0502x over previous
"""Trainium2 Bass kernel for nn_CircumpunctAttention_17136919511703.

Sharding: 16 heads tensor-parallel over 8 cores (2 heads/core); W_out
row-parallel with the cross-core partial sum done on the host.

Math simplifications (validated vs the jax reference):
  - attn = softmax(s)*ap*ex renormalized by (sum + 1e-8): the aperture
    gate ap cancels exactly up to the 1e-8 term, so it is dropped.
    converged = (e' @ v) / (e' @ 1) with e'_st = ex_s * exp(scoresT_st)
    (sender gate folded into vpp).
  - no softmax max-subtraction (scores are bounded, |s| < 4).
  - the per-head "aperture chamber" (valve in/out, phase rotation, chi)
    is a per-head linear map on the head dim -> folded into W_out on the
    host: W'_h = c_h * Wout_h @ R_h.

Schedule (engines execute strictly in program order, so emission order
is the schedule):
  wave(half): i/o/v/ex projections for one T-half, 4 concurrent PSUM
    accumulation streams paced to the xT tile DMAs; ex is computed in
    natural [t, h] orientation (tiny free dim) so no transposes or M=2
    waves are needed; v transposed to natural and gated by ex into vpp.
  attention: 4 j-blocks of 512 t-cols; per block a software-pipelined
    i-loop emits scores(i+2)+exp(i+2) BEFORE PT(i) so the PE never
    waits on the ACT exp; pt accumulators [65, 512] live in one 2-bank
    PSUM slot.  post(j) normalizes via DVE reciprocal + Pool
    partition-broadcast + DVE multiply (no PE, no ACT).  Final output
    matmuls for block j are interleaved into block j+1's loop; y tiles
    staged via Pool-engine copies and DMA'd out per 128-row slice.
PSUM budget: tags sc(2x[128,1024]) + pt(1x[128,1024]) + py(2x[128,512])
  = exactly 8 banks, shared by all phases.
"""

import math
import os
from contextlib import ExitStack

import numpy as np

import concourse.bass as bass
import concourse.mybir as mybir
import concourse.tile as tile
from concourse import bacc
from concourse.bass_utils import run_bass_kernel_spmd
from concourse.masks import make_identity

T, D, H, DH = 2048, 1024, 16, 64
NCORES = 8
HPC = H // NCORES          # heads per core = 2
KW = HPC * DH              # per-core head width = 128
SCALE = math.sqrt(DH)
DT = D // 128              # d tiles = 8
TT = T // 128              # t/s tiles = 16
F32 = mybir.dt.float32
F32R = mybir.dt.float32r
BF16 = mybir.dt.bfloat16
AF = mybir.ActivationFunctionType

_CACHE = {}
LAST_RESULTS = None


def _build_nc():
    nc = bacc.Bacc()
    xT = nc.declare_dram_parameter("xT", [128, DT, T], BF16, isOutput=False)
    wiT = nc.declare_dram_parameter("wiT", [128, DT, KW], BF16, isOutput=False)
    woT = nc.declare_dram_parameter("woT", [128, DT, KW], BF16, isOutput=False)
    wvT = nc.declare_dram_parameter("wvT", [128, DT, KW], BF16, isOutput=False)
    weT = nc.declare_dram_parameter("weT", [128, DT, HPC], BF16, isOutput=False)
    webn = nc.declare_dram_parameter("webn", [128, TT * HPC], F32, isOutput=False)
    wpT = nc.declare_dram_parameter("wpT", [KW, D], F32R, isOutput=False)
    y = nc.declare_dram_parameter("y", [T, D], BF16, isOutput=True)

    with tile.TileContext(nc) as tc, ExitStack() as ctx:
        _body(ctx, tc, xT, wiT, woT, wvT, weT, webn, wpT, y)
    nc.compile()
    return nc


def _body(ctx, tc, xT, wiT, woT, wvT, weT, webn, wpT, y):
    nc = tc.nc
    P = 128
    HB = 512                   # j-block width
    NJ = T // HB               # 4 j-blocks

    const = ctx.enter_context(tc.tile_pool(name="const", bufs=1))
    persist = ctx.enter_context(tc.tile_pool(name="persist", bufs=1))
    eTp = ctx.enter_context(tc.tile_pool(name="eTp", bufs=4))
    stage = ctx.enter_context(tc.tile_pool(name="stage", bufs=2))
    psum = ctx.enter_context(tc.tile_pool(name="psum", bufs=1, space="PSUM"))

    # ---- DMA issue order == consumption order ----
    wiT_sb = const.tile([P, DT, KW], BF16)
    woT_sb = const.tile([P, DT, KW], BF16)
    wvT_sb = const.tile([P, DT, KW], BF16)
    weT_sb = const.tile([P, DT, HPC], BF16)
    xT_sb = persist.tile([P, DT, T], BF16)
    wpT_sb = const.tile([KW, D], F32R)
    webn_sb = const.tile([P, TT * HPC], F32)
    nc.sync.dma_start(out=wiT_sb, in_=wiT[:, :, :])
    nc.sync.dma_start(out=xT_sb[:, 0, 0:1024], in_=xT[:, 0, 0:1024])
    nc.sync.dma_start(out=woT_sb, in_=woT[:, :, :])
    nc.sync.dma_start(out=wvT_sb, in_=wvT[:, :, :])
    nc.sync.dma_start(out=weT_sb, in_=weT[:, :, :])
    for a in range(1, DT):
        nc.sync.dma_start(out=xT_sb[:, a, 0:1024], in_=xT[:, a, 0:1024])
    nc.sync.dma_start(out=webn_sb, in_=webn[:, :])
    for a in range(DT):
        nc.sync.dma_start(out=xT_sb[:, a, 1024:T], in_=xT[:, a, 1024:T])
    nc.sync.dma_start(out=wpT_sb, in_=wpT[:, :])

    ident = const.tile([P, P], F32)
    make_identity(nc, ident)
    onep = const.tile([P, 1], F32)
    nc.vector.memset(onep, 1.0)

    innerT = persist.tile([P, T], F32R)
    outerT = persist.tile([P, T], F32R)
    vT = persist.tile([P, T], F32)
    vpp = persist.tile([P, TT, HPC, 65], F32R)
    # col 64 must be 1.0 (ones row for the E sums); cols 0:64 are fully
    # overwritten by the v copies, so a contiguous whole-tile memset works
    nc.vector.memset(vpp.bitcast(F32), 1.0)
    exz = persist.tile([P, TT * HPC], F32)   # sigmoid(ex logits), [t, i*2+h]
    convT = persist.tile([KW, T], F32R)

    def xchunk(a, c):
        # c indexes 512-wide chunks of T
        return xT_sb[:, a, c * 512:(c + 1) * 512]

    def wave(half):
        psI = psum.tile([P, 1024], F32, tag="sc", bufs=2, name="psI")
        psO = psum.tile([P, 1024], F32, tag="sc", bufs=2, name="psO")
        psV = psum.tile([P, 1024], F32, tag="pt", bufs=1, name="psV")
        for a in range(DT):
            st, sp = (a == 0), (a == DT - 1)
            for j2 in range(2):
                c = half * 2 + j2
                nc.tensor.matmul(
                    psI[:, j2 * 512:(j2 + 1) * 512],
                    lhsT=wiT_sb[:, a, :], rhs=xchunk(a, c),
                    start=st, stop=sp)
                nc.tensor.matmul(
                    psO[:, j2 * 512:(j2 + 1) * 512],
                    lhsT=woT_sb[:, a, :], rhs=xchunk(a, c),
                    start=st, stop=sp)
                nc.tensor.matmul(
                    psV[:, j2 * 512:(j2 + 1) * 512],
                    lhsT=wvT_sb[:, a, :], rhs=xchunk(a, c),
                    start=st, stop=sp)
        hs = slice(half * 1024, (half + 1) * 1024)
        nc.vector.tensor_copy(out=innerT[:, hs], in_=psI)
        nc.vector.tensor_copy(out=outerT[:, hs], in_=psO)
        nc.scalar.copy(out=vT[:, hs], in_=psV)

        # ex logits per t-tile (natural [t, h] layout, all tiny); one
        # accumulation group per PSUM bank (start=True resets bank-wide)
        for k in range(8):
            m = half * 8 + k
            pex = psum.tile([P, HPC], F32, tag="py", bufs=2, name="pex")
            for a in range(DT):
                nc.tensor.matmul(
                    pex,
                    lhsT=xT_sb[:, a, m * P:(m + 1) * P],
                    rhs=weT_sb[:, a, :],
                    start=(a == 0), stop=(a == DT - 1))
            nc.vector.tensor_add(out=exz[:, HPC * m:HPC * (m + 1)], in0=pex,
                                 in1=webn_sb[:, HPC * m:HPC * (m + 1)])
        cs = slice(half * 16, (half + 1) * 16)
        nc.scalar.activation(out=exz[:, cs], in_=exz[:, cs],
                             func=AF.Exp, scale=-1.0)
        nc.vector.tensor_scalar(out=exz[:, cs], in0=exz[:, cs],
                                scalar1=onep, scalar2=None,
                                op0=mybir.AluOpType.add)
        nc.vector.reciprocal(out=exz[:, cs], in_=exz[:, cs])

        # v -> natural per-head layout, gated by ex
        pstr = psum.tile([P, 1024], F32, tag="sc", bufs=2, name="pstr")
        for k in range(8):
            i = half * 8 + k
            nc.tensor.transpose(
                pstr[:, k * P:(k + 1) * P], vT[:, i * P:(i + 1) * P], ident)
        psr = pstr.rearrange("p (k c) -> p k c", c=P)
        g8 = slice(half * 8, (half + 1) * 8)
        for h in range(HPC):
            nc.vector.tensor_copy(
                out=vpp[:, g8, h, 0:DH],
                in_=psr[:, :, h * DH:(h + 1) * DH])
        for k in range(8):
            i = half * 8 + k
            for h in range(HPC):
                nc.vector.tensor_scalar_mul(
                    out=vpp[:, i, h, :], in0=vpp[:, i, h, :],
                    scalar1=exz[:, HPC * i + h:HPC * i + h + 1])

    wave(0)
    wave(1)

    # ---- attention: one flat software-pipelined loop over k = j*16+i ----
    NK = NJ * TT               # 64

    def sc_exp(k):
        i, j = k % TT, k // TT
        sc = psum.tile([P, 1024], F32, tag="sc", bufs=2, name="sc")
        nc.tensor.matmul(
            sc[:, 0:512],
            lhsT=outerT[0:DH, i * P:(i + 1) * P].bitcast(F32R),
            rhs=innerT[0:DH, j * HB:(j + 1) * HB].bitcast(F32R),
            start=True, stop=True)
        nc.tensor.matmul(
            sc[:, 512:1024],
            lhsT=outerT[DH:KW, i * P:(i + 1) * P].bitcast(F32R),
            rhs=innerT[DH:KW, j * HB:(j + 1) * HB].bitcast(F32R),
            start=True, stop=True)
        eT = eTp.tile([P, 1024], F32R, tag="e", name="eT")
        nc.scalar.activation(out=eT, in_=sc, func=AF.Exp, scale=1.0)
        return eT

    def emit_final(m, tail=False):
        # y partial for t-tile m; PSUM->SBUF copies on DVE (ACT helps in the
        # tail, once the exp stream is done), then DMA
        y_sb = stage.tile([P, D], BF16, tag="y", name="y_sb")
        for o in range(2):
            py = psum.tile([P, 512], F32, tag="py", bufs=2, name="py")
            nc.tensor.matmul(
                py,
                lhsT=convT[:, m * P:(m + 1) * P].bitcast(F32R),
                rhs=wpT_sb[:, o * 512:(o + 1) * 512].bitcast(F32R),
                start=True, stop=True)
            if tail and o == 0:
                nc.scalar.copy(out=y_sb[:, 0:512], in_=py)
            else:
                nc.vector.tensor_copy(out=y_sb[:, o * 512:(o + 1) * 512],
                                      in_=py)
            nc.sync.dma_start(out=y[m * P:(m + 1) * P, o * 512:(o + 1) * 512],
                              in_=y_sb[:, o * 512:(o + 1) * 512])

    def post(j, pt, tail=False):
        # conv = PT[0:64] / E, E in row 64; no PE, no ACT.  One fast DVE
        # copy frees the pt PSUM slot; the rest runs from SBUF off the
        # critical path.  In the tail the slot is never reused, so read
        # the PSUM directly and skip the copy.
        if tail:
            pt_sb = pt
        else:
            pt_sb = stage.tile([65, 1024], F32, tag="ptsb", name="pt_sb")
            nc.vector.tensor_copy(out=pt_sb, in_=pt)
        f_row = stage.tile([1, 1024], F32R, tag="f", name="f_row")
        with nc.allow_low_precision(reason="f32r is a 4-byte container"):
            nc.vector.reciprocal(out=f_row, in_=pt_sb[64:65, 0:1024])
        fbc = stage.tile([DH, 1024], F32R, tag="fb", name="fbc")
        nc.gpsimd.partition_broadcast(fbc, f_row, channels=DH)
        jc = slice(j * HB, (j + 1) * HB)
        nc.vector.tensor_mul(out=convT[0:DH, jc], in0=pt_sb[0:DH, 0:512],
                             in1=fbc[:, 0:512])
        nc.vector.tensor_mul(out=convT[DH:KW, jc], in0=pt_sb[0:DH, 512:1024],
                             in1=fbc[:, 512:1024])

    y_pend = []                # finals not yet emitted: list of m
    pt = None
    eTs = {0: sc_exp(0), 1: sc_exp(1)}
    for k in range(NK):
        i, j = k % TT, k // TT
        if k + 2 < NK:
            eTs[k + 2] = sc_exp(k + 2)
        if i == 0:
            pt = psum.tile([65, 1024], F32, tag="pt", bufs=1, name="pt")
        if i in (4, 7, 10, 13) and y_pend:
            emit_final(y_pend.pop(0))
        eT = eTs.pop(k)
        nc.tensor.matmul(
            pt[0:65, 0:512],
            lhsT=vpp[:, i, 0, :].bitcast(F32R),
            rhs=eT[:, 0:512],
            start=(i == 0), stop=(i == TT - 1))
        nc.tensor.matmul(
            pt[0:65, 512:1024],
            lhsT=vpp[:, i, 1, :].bitcast(F32R),
            rhs=eT[:, 512:1024],
            start=(i == 0), stop=(i == TT - 1))
        if i == TT - 1:
            post(j, pt, tail=(j == NJ - 1))
            y_pend.extend(range(j * 4, (j + 1) * 4))

    for m in y_pend:
        emit_final(m, tail=True)


def _sigmoid(z):
    return 1.0 / (1.0 + np.exp(-z))


def _prep_in_maps(inputs):
    x = np.ascontiguousarray(np.asarray(inputs["x"], np.float32)[0])  # [T, D]
    xT = np.ascontiguousarray(x.T)                                    # [D, T]
    Wi = np.asarray(inputs["Wi_w"], np.float32).reshape(H, DH, D) / SCALE
    Wo = np.asarray(inputs["Wo_w"], np.float32).reshape(H, DH, D)
    Wv = np.asarray(inputs["Wv_w"], np.float32).reshape(H, DH, D)
    We = np.asarray(inputs["We_w"], np.float32)                       # [H, D]
    We_b = np.asarray(inputs["We_b"], np.float32)                     # [H]
    Wout = np.asarray(inputs["Wout_w"], np.float32)                   # [D, D]
    beta = np.asarray(inputs["beta"], np.float32)
    iv = np.asarray(inputs["iv"], np.float32)
    ov = np.asarray(inputs["ov"], np.float32)
    chi = np.asarray(inputs["chi"], np.float32)

    # chamber folded into Wout: W'_h = c_h * Wout_h @ R_h
    ang = np.float32(math.pi) * _sigmoid(beta)
    c_h = _sigmoid(iv) * _sigmoid(ov) * np.tanh(chi)                  # [H]
    cos_a, sin_a = np.cos(ang), np.sin(ang)
    HALF = DH // 2
    Wp = np.zeros((H, D, DH), np.float32)
    for h in range(H):
        Wh = Wout[:, h * DH:(h + 1) * DH]
        Wp[h][:, :HALF] = c_h[h] * (Wh[:, :HALF] * cos_a[h] + Wh[:, HALF:] * sin_a[h])
        Wp[h][:, HALF:] = c_h[h] * (-Wh[:, :HALF] * sin_a[h] + Wh[:, HALF:] * cos_a[h])

    def dtile(arr):  # [D, X] -> [128, DT, X] (d-tile-major, partition-contig)
        return np.ascontiguousarray(
            arr.reshape(DT, 128, arr.shape[1]).transpose(1, 0, 2))

    import ml_dtypes
    bf16 = ml_dtypes.bfloat16
    xTr = dtile(xT).astype(bf16)
    in_maps = []
    for c in range(NCORES):
        hs = slice(HPC * c, HPC * (c + 1))
        wiT = dtile(Wi[hs].reshape(KW, D).T).astype(bf16)
        woT = dtile(Wo[hs].reshape(KW, D).T).astype(bf16)
        wvT = dtile(Wv[hs].reshape(KW, D).T).astype(bf16)
        weT = dtile(We[hs].T).astype(bf16)
        webn = np.ascontiguousarray(np.broadcast_to(
            np.tile(We_b[hs], TT), (128, TT * HPC)).astype(np.float32))
        wpT = np.ascontiguousarray(
            Wp[hs].transpose(0, 2, 1).reshape(KW, D))                 # [128, D]
        in_maps.append(dict(xT=xTr, wiT=wiT, woT=woT, wvT=wvT,
                            weT=weT, webn=webn, wpT=wpT))
    return in_maps


def kernel(**inputs):
    global LAST_RESULTS
    if "nc" not in _CACHE:
        _CACHE["nc"] = _build_nc()
    nc = _CACHE["nc"]
    in_maps = _prep_in_maps(inputs)
    trace = os.environ.get("CIRC_TRACE", "") not in ("", "0")
    res = run_bass_kernel_spmd(
        nc, in_maps, core_ids=list(range(NCORES)), trace=trace)
    LAST_RESULTS = res
    y = res.results[0]["y"].astype(np.float32)
    for c in range(1, NCORES):
        y = y + res.results[c]["y"].astype(np.float32)
    return y.reshape(1, T, D)
